# revision 1
# baseline (speedup 1.0000x reference)
"""DGRU cell fused kernel for Trainium2, data-parallel over 8 NeuronCores.

Reference computation (per batch row, d=512):
    inp  = LN([x, h]) * mask                       [B, 2d]
    g    = inp @ Wg.T + bg                         [B, 5d]
    rx, rh = sigmoid(g[:, :d]), sigmoid(g[:, d:2d])
    z    = softmax over the three chunks g[:, 2d:5d]
    inp2 = LN2([x*rx, h*rh]) * mask
    u    = tanh(inp2 @ Wu.T + bu)
    out  = x*z0 + h*z1 + u*z2

Device strategy (per core, 4096 rows):
  - batch rows on SBUF partitions, 32 row-tiles of 128
  - LN affine (w,b) and dropout mask folded into Wg/Wu + biases on host
  - LN normalize on DVE (fused (x-mu)*rstd tensor_scalar); rstd via
    bit-trick + 2 Newton iterations on DVE (ACT Rsqrt is banned and would
    force an activation-table switch)
  - normalized activations PE-transposed (fp32r) into [feat, batch] layout,
    matmuls run in float32r (fp32 storage, 1 cycle/row at N=512)
  - biases folded into the matmul as an extra K=1 accumulation row
    (stationary = ones[1,128]); softmax-invariance removes the bias on the
    third z chunk entirely
  - sigmoid via tanh identity (sigmoid(t) = (1+tanh(t/2))/2) so every ACT
    call (tanh/exp/copy) lives in the single `exp_and_others` table set;
    the resulting global factor 2 on inp2 cancels in LN2 (eps scaled 4x)
  - softmax without max-subtraction (gate magnitudes are ~N(0,0.6)), exact
    DVE reciprocal for the denominator
"""

import contextlib

import numpy as np

import concourse.bass as bass
import concourse.mybir as mybir
import concourse.tile as tile
from concourse import bacc
from concourse.bass_utils import run_bass_kernel_spmd
from concourse.masks import make_identity

N_CORES = 8
B = 32768
D = 512
D2 = 2 * D          # 1024 = contraction dim
D5 = 5 * D          # 2560 = gate dim
B_LOC = B // N_CORES
P = 128
NT = B_LOC // P     # row-tiles per core
KC = D2 // P        # K chunks (8)
EPS = 1e-5
GATE_BIAS = 0.0

F32 = mybir.dt.float32
F32R = mybir.dt.float32r
I32 = mybir.dt.int32
AF = mybir.ActivationFunctionType
OP = mybir.AluOpType

BF16 = mybir.dt.bfloat16
MAGIC = 0x5F3759DF  # fast inverse sqrt seed


def _rsqrt(nc, pool, var, eps, magic, tag):
    """r = 1/sqrt(var + eps) on DVE: bit-trick seed + 2 Newton steps."""
    v = pool.tile([P, 1], F32, tag=f"v_{tag}")
    nc.vector.tensor_scalar_add(v, var, float(eps))
    y = pool.tile([P, 1], F32, tag=f"y_{tag}")
    # y0 = bitcast(MAGIC - (bitcast_i32(v) >> 1)); int immediates are lowered
    # as fp32, so both int ops use tiny constant tiles instead.
    nc.vector.tensor_tensor(y.bitcast(I32), v.bitcast(I32), magic[:, 1:2],
                            op=OP.logical_shift_right)
    nc.vector.tensor_tensor(y.bitcast(I32), magic[:, 0:1], y.bitcast(I32),
                            op=OP.subtract)
    a = pool.tile([P, 1], F32, tag=f"a_{tag}")
    for _ in range(2):
        nc.vector.tensor_tensor(a, y, y, op=OP.mult)
        nc.vector.tensor_tensor(a, a, v, op=OP.mult)
        nc.vector.tensor_scalar(a, a, -0.5, 1.5, op0=OP.mult, op1=OP.add)
        nc.vector.tensor_tensor(y, y, a, op=OP.mult)
    return y


def _build(repeat=1, no_mm=False, no_tp=False, no_ln=False, no_epi=False,
           no_aug=False, mm_corder=False, no_recip=False, dve_bias=False,
           bf16=False, dma_tp=False, pools2=False, pipe=False):
    nc = bacc.Bacc("TRN2", target_bir_lowering=False, debug=False,
                   num_devices=N_CORES)
    x_d = nc.declare_dram_parameter("x", [B_LOC, D], F32, isOutput=False)
    h_d = nc.declare_dram_parameter("h", [B_LOC, D], F32, isOutput=False)
    wgt_d = nc.declare_dram_parameter("wgt", [D2, D5], F32, isOutput=False)
    gb_d = nc.declare_dram_parameter("gb", [1, 4 * D], F32, isOutput=False)
    wut_d = nc.declare_dram_parameter("wut", [D2, D], F32, isOutput=False)
    ub_d = nc.declare_dram_parameter("ub", [1, D], F32, isOutput=False)
    out_d = nc.declare_dram_parameter("out", [B_LOC, D], F32, isOutput=True)

    with tile.TileContext(nc) as tc:
        if no_mm or no_tp or no_ln or no_epi:
            tc.race_detector_enabled = False
        with (
            tc.tile_pool(name="static", bufs=1) as static,
            tc.tile_pool(name="io", bufs=4 if pools2 else 3) as io,
            tc.tile_pool(name="work", bufs=2) as work,
            tc.tile_pool(name="small", bufs=4 if pools2 else 3) as small,
            tc.tile_pool(name="mm", bufs=6, space="PSUM") as mm,
            tc.tile_pool(name="tp", bufs=2 if pools2 else 1, space="PSUM") as tp,
        ):
            # ---- static tiles ----
            MMDT = BF16 if bf16 else F32R
            wgt = static.tile([P, KC * D5], MMDT)      # WgT, 8 chunks of [128, 2560]
            wut = static.tile([P, KC * D], MMDT)       # WuT, 8 chunks of [128, 512]
            if bf16:
                with tc.tile_pool(name="stage", bufs=2) as stage:
                    for c in range(KC):
                        sg = stage.tile([P, D5], F32, tag="sg")
                        nc.sync.dma_start(out=sg, in_=wgt_d[c * P:(c + 1) * P, :])
                        nc.vector.tensor_copy(wgt[:, c * D5:(c + 1) * D5], sg)
                    for c in range(KC):
                        su = stage.tile([P, D], F32, tag="su")
                        nc.sync.dma_start(out=su, in_=wut_d[c * P:(c + 1) * P, :])
                        nc.vector.tensor_copy(wut[:, c * D:(c + 1) * D], su)
            else:
                for c in range(KC):
                    nc.sync.dma_start(out=wgt[:, c * D5:(c + 1) * D5],
                                      in_=wgt_d[c * P:(c + 1) * P, :].bitcast(F32R))
                for c in range(KC):
                    nc.sync.dma_start(out=wut[:, c * D:(c + 1) * D],
                                      in_=wut_d[c * P:(c + 1) * P, :].bitcast(F32R))
            if dve_bias:
                bgb = static.tile([P, 4 * D], F32)
                nc.sync.dma_start(out=bgb, in_=gb_d[:, :].to_broadcast([P, 4 * D]))
                ubb = static.tile([P, D], F32)
                nc.sync.dma_start(out=ubb, in_=ub_d[:, :].to_broadcast([P, D]))
            elif bf16:
                gb_f = static.tile([1, 4 * D], F32)
                nc.sync.dma_start(out=gb_f, in_=gb_d[:, :])
                ub_f = static.tile([1, D], F32)
                nc.sync.dma_start(out=ub_f, in_=ub_d[:, :])
                gb = static.tile([1, 4 * D], BF16)
                nc.vector.tensor_copy(gb, gb_f)
                ub = static.tile([1, D], BF16)
                nc.vector.tensor_copy(ub, ub_f)
            else:
                gb = static.tile([1, 4 * D], F32R)
                nc.sync.dma_start(out=gb, in_=gb_d[:, :].bitcast(F32R))
                ub = static.tile([1, D], F32R)
                nc.sync.dma_start(out=ub, in_=ub_d[:, :].bitcast(F32R))
            ones_f = static.tile([1, P], F32)
            nc.vector.memset(ones_f, 1.0)
            ones_row = static.tile([1, P], MMDT)
            nc.vector.tensor_copy(ones_row, ones_f)
            ident_f = static.tile([P, P], F32)
            make_identity(nc, ident_f)
            ident = static.tile([P, P], MMDT)
            nc.vector.tensor_copy(ident, ident_f)
            magic = static.tile([P, 2], I32)   # col0 = seed, col1 = shift amount
            nc.vector.memset(magic[:, 0:1], MAGIC)
            nc.vector.memset(magic[:, 1:2], 1)

            def tp_transpose(src_t, dst, nm):
                """PE-transpose src [128,1024] into dst via 2 one-bank psum
                tiles; copies split across ACT and DVE."""
                for half in (0, 1):
                    tph = tp.tile([P, D], MMDT, tag="tp", name=f"tp_{nm}{half}")
                    for c in range(4):
                        cc = half * 4 + c
                        nc.tensor.transpose(tph[:, c * P:(c + 1) * P],
                                            src_t[:, cc * P:(cc + 1) * P], ident)
                    if half == 0:
                        nc.scalar.copy(dst[:, :D], tph)
                    else:
                        nc.vector.tensor_copy(dst[:, D:], tph)

            def ln_chain(src_t, eps, nm):
                st = small.tile([P, 2, 6], F32, tag=f"st_{nm}", name=f"st_{nm}")
                nc.vector.bn_stats(st[:, 0, :], src_t[:, :D])
                nc.vector.bn_stats(st[:, 1, :], src_t[:, D:])
                mvv = small.tile([P, 2], F32, tag=f"mv_{nm}", name=f"mv_{nm}")
                nc.vector.bn_aggr(mvv, st)
                rr = _rsqrt(nc, small, mvv[:, 1:2], eps, magic, nm)
                return mvv, rr

            def front(r):
                rows = slice(r * P, (r + 1) * P)
                inp = io.tile([P, D2], F32, tag="inp", name="inp")
                nc.sync.dma_start(out=inp[:, :D], in_=x_d[rows, :])
                nc.sync.dma_start(out=inp[:, D:], in_=h_d[rows, :])
                mvv, r1 = ln_chain(inp, EPS, "r1")
                normed = work.tile([P, D2], MMDT, tag="normed", name="normed")
                nc.vector.tensor_scalar(normed, inp, mvv[:, 0:1], r1,
                                        op0=OP.subtract, op1=OP.mult)
                inpT = work.tile([P, D2], MMDT, tag="inpT", bufs=3, name="inpT")
                tp_transpose(normed, inpT, "f")
                t12 = work.tile([P, D2], F32, tag="t12", name="t12")
                e = work.tile([P, 3 * D], F32, tag="e", name="e")
                for n in range(5):
                    g = mm.tile([P, D], F32, tag="mmtile", name=f"g{n}")
                    for c in range(KC):
                        nc.tensor.matmul(
                            g,
                            lhsT=inpT[:, c * P:(c + 1) * P],
                            rhs=wgt[:, c * D5 + n * D: c * D5 + (n + 1) * D],
                            start=(c == 0), stop=(c == KC - 1 and n == 4),
                        )
                    if n < 4:
                        nc.tensor.matmul(
                            g, lhsT=ones_row, rhs=gb[:, n * D:(n + 1) * D],
                            start=False, stop=True,
                        )
                    # consume the psum tile immediately so the pool turns over
                    if n == 0:
                        nc.scalar.activation(t12[:, :D], g, AF.Tanh, scale=0.5)
                    elif n == 1:
                        nc.scalar.activation(t12[:, D:], g, AF.Tanh, scale=0.5)
                    else:
                        j = n - 2
                        nc.scalar.activation(e[:, j * D:(j + 1) * D], g, AF.Exp)
                return rows, inp, t12, e

            def back(stv):
                rows, inp, t12, e = stv
                nc.vector.scalar_tensor_tensor(t12, t12, 1.0, inp,
                                               op0=OP.add, op1=OP.mult)
                mv2, r2 = ln_chain(t12, 4.0 * EPS, "r2")
                normed2 = work.tile([P, D2], MMDT, tag="normed", name="normed2")
                nc.vector.tensor_scalar(normed2, t12, mv2[:, 0:1], r2,
                                        op0=OP.subtract, op1=OP.mult)
                inp2T = work.tile([P, D2], MMDT, tag="inpT", bufs=3, name="inp2T")
                tp_transpose(normed2, inp2T, "b")
                ups = mm.tile([P, D], F32, tag="mmtile", name="ups")
                for c in range(KC):
                    nc.tensor.matmul(
                        ups, lhsT=inp2T[:, c * P:(c + 1) * P],
                        rhs=wut[:, c * D:(c + 1) * D],
                        start=(c == 0), stop=False,
                    )
                nc.tensor.matmul(ups, lhsT=ones_row, rhs=ub,
                                 start=False, stop=True)
                u = work.tile([P, D], F32, tag="u", name="u")
                nc.scalar.activation(u, ups, AF.Tanh)

                s = work.tile([P, D], F32, tag="s", name="s")
                nc.gpsimd.tensor_tensor(s, e[:, :D], e[:, D:2 * D], op=OP.add)
                nc.gpsimd.tensor_tensor(s, s, e[:, 2 * D:], op=OP.add)
                rs = work.tile([P, D], F32, tag="rs", name="rs")
                nc.vector.reciprocal(rs, s)
                m1 = work.tile([P, D], F32, tag="m1", name="m1")
                nc.vector.tensor_tensor(m1, inp[:, :D], e[:, :D], op=OP.mult)
                m2 = work.tile([P, D], F32, tag="m2", name="m2")
                nc.gpsimd.tensor_tensor(m2, inp[:, D:], e[:, D:2 * D], op=OP.mult)
                m3 = work.tile([P, D], F32, tag="m3", name="m3")
                nc.gpsimd.tensor_tensor(m3, u, e[:, 2 * D:], op=OP.mult)
                nc.vector.tensor_tensor(m1, m1, m2, op=OP.add)
                nc.vector.tensor_tensor(m1, m1, m3, op=OP.add)
                nc.vector.tensor_tensor(m1, m1, rs, op=OP.mult)
                nc.sync.dma_start(out=out_d[rows, :], in_=m1)

            loop_cm = tc.For_i(0, repeat, 1) if repeat > 1 else contextlib.nullcontext()
            with loop_cm:
              if pipe:
                prev = None
                for r in range(NT):
                    cur = front(r)
                    if prev is not None:
                        back(prev)
                    prev = cur
                back(prev)
              else:
                for r in range(NT):
                    rows = slice(r * P, (r + 1) * P)
                    inp = io.tile([P, D2], F32, tag="inp")
                    nc.sync.dma_start(out=inp[:, :D], in_=x_d[rows, :])
                    nc.sync.dma_start(out=inp[:, D:], in_=h_d[rows, :])
                    xv, hv = inp[:, :D], inp[:, D:]

                    # ---- LN1 ----
                    normed = work.tile([P, D2], MMDT, tag="normed",
                                       bufs=3 if pools2 else None)
                    if not no_ln:
                        st = small.tile([P, 2, 6], F32, tag="st")
                        nc.vector.bn_stats(st[:, 0, :], inp[:, :D])
                        nc.vector.bn_stats(st[:, 1, :], inp[:, D:])
                        mv = small.tile([P, 2], F32, tag="mv")
                        nc.vector.bn_aggr(mv, st)
                        r1 = _rsqrt(nc, small, mv[:, 1:2], EPS, magic, "r1")
                        nc.vector.tensor_scalar(normed, inp, mv[:, 0:1], r1,
                                                op0=OP.subtract, op1=OP.mult)
                    else:
                        nc.vector.tensor_copy(normed, inp)

                    # ---- transpose LN1 out, 8 chunks of [128,128] ----
                    inpT = work.tile([P, D2], MMDT, tag="inpT",
                                     bufs=3 if pools2 else None)
                    if no_tp:
                        nc.vector.tensor_copy(inpT, normed)
                    elif dma_tp:
                        for c in range(KC):
                            cs = slice(c * P, (c + 1) * P)
                            nc.sync.dma_start_transpose(inpT[:, cs], normed[:, cs])
                    elif pools2:
                        for half, eng in ((0, nc.scalar), (1, nc.vector)):
                            tph = tp.tile([P, D], MMDT, tag="tp", name=f"tp{half}")
                            for c in range(4):
                                cc = half * 4 + c
                                nc.tensor.transpose(
                                    tph[:, c * P:(c + 1) * P],
                                    normed[:, cc * P:(cc + 1) * P], ident)
                            if half == 0:
                                nc.scalar.copy(inpT[:, :D], tph)
                            else:
                                nc.vector.tensor_copy(inpT[:, D:], tph)
                    else:
                        tpt = tp.tile([P, D2], MMDT, tag="tp")
                        for c in range(KC):
                            cs = slice(c * P, (c + 1) * P)
                            nc.tensor.transpose(tpt[:, cs], normed[:, cs], ident)
                        nc.scalar.copy(inpT[:, :D], tpt[:, :D])
                        nc.vector.tensor_copy(inpT[:, D:], tpt[:, D:])

                    # ---- gates matmul: 5 psum tiles of [128, 512] ----
                    gps = [mm.tile([P, D], F32, tag="mmtile", name=f"g{i}") for i in range(5)]
                    if no_mm:
                        for g in gps:
                            nc.vector.memset(g[:, :1], 1.0)
                    if not no_mm:
                        order = ([(c, n) for c in range(KC) for n in range(5)]
                                 if mm_corder else
                                 [(c, n) for n in range(5) for c in range(KC)])
                        for c, n in order:
                            nc.tensor.matmul(
                                gps[n],
                                lhsT=inpT[:, c * P:(c + 1) * P],
                                rhs=wgt[:, c * D5 + n * D: c * D5 + (n + 1) * D],
                                start=(c == 0),
                                stop=(c == KC - 1 and (n == 4 or no_aug or dve_bias)),
                            )
                        if not no_aug and not dve_bias:
                            for n in range(4):
                                nc.tensor.matmul(
                                    gps[n], lhsT=ones_row,
                                    rhs=gb[:, n * D:(n + 1) * D],
                                    start=False, stop=True,
                                )

                    # ---- rx/rh via tanh(g/2); z numerators via exp ----
                    if no_epi:
                        m1 = work.tile([P, D], F32, tag="m1")
                        nc.vector.memset(m1[:, :1], 1.0)
                        nc.sync.dma_start(out=out_d[rows, :], in_=m1)
                        continue
                    t12 = work.tile([P, D2], F32, tag="t12")
                    e = work.tile([P, 3 * D], F32, tag="e")
                    if dve_bias:
                        s01 = work.tile([P, D2], F32, tag="s01")
                        nc.vector.tensor_tensor(s01[:, :D], gps[0], bgb[:, :D],
                                                op=OP.add)
                        nc.vector.tensor_tensor(s01[:, D:], gps[1],
                                                bgb[:, D:2 * D], op=OP.add)
                        nc.scalar.activation(t12, s01, AF.Tanh, scale=0.5)
                        nc.vector.tensor_tensor(e[:, :D], gps[2],
                                                bgb[:, 2 * D:3 * D], op=OP.add)
                        nc.vector.tensor_tensor(e[:, D:2 * D], gps[3],
                                                bgb[:, 3 * D:4 * D], op=OP.add)
                        nc.scalar.activation(e[:, :2 * D], e[:, :2 * D], AF.Exp)
                        nc.scalar.activation(e[:, 2 * D:], gps[4], AF.Exp)
                    else:
                        nc.scalar.activation(t12[:, :D], gps[0], AF.Tanh, scale=0.5)
                        nc.scalar.activation(t12[:, D:], gps[1], AF.Tanh, scale=0.5)
                        for j in range(3):
                            nc.scalar.activation(e[:, j * D:(j + 1) * D],
                                                 gps[2 + j], AF.Exp)

                    # inp2 = (1 + tanh) * inp = 2*[x*rx, h*rh] (factor cancels in LN2)
                    nc.vector.scalar_tensor_tensor(t12, t12, 1.0, inp,
                                                   op0=OP.add, op1=OP.mult)

                    # ---- LN2 (eps*4 compensates the factor-2 scale) ----
                    normed2 = work.tile([P, D2], MMDT, tag="normed",
                                        bufs=3 if pools2 else None)
                    if not no_ln:
                        st2 = small.tile([P, 2, 6], F32, tag="st2")
                        nc.vector.bn_stats(st2[:, 0, :], t12[:, :D])
                        nc.vector.bn_stats(st2[:, 1, :], t12[:, D:])
                        mv2 = small.tile([P, 2], F32, tag="mv2")
                        nc.vector.bn_aggr(mv2, st2)
                        r2 = _rsqrt(nc, small, mv2[:, 1:2], 4.0 * EPS, magic, "r2")
                        nc.vector.tensor_scalar(normed2, t12, mv2[:, 0:1], r2,
                                                op0=OP.subtract, op1=OP.mult)
                    else:
                        nc.vector.tensor_copy(normed2, t12)

                    inp2T = work.tile([P, D2], MMDT, tag="inpT",
                                      bufs=3 if pools2 else None)
                    if no_tp:
                        nc.vector.tensor_copy(inp2T, normed2)
                    elif dma_tp:
                        for c in range(KC):
                            cs = slice(c * P, (c + 1) * P)
                            nc.sync.dma_start_transpose(inp2T[:, cs], normed2[:, cs])
                    elif pools2:
                        for half, eng in ((0, nc.scalar), (1, nc.vector)):
                            tph = tp.tile([P, D], MMDT, tag="tp", name=f"tp2{half}")
                            for c in range(4):
                                cc = half * 4 + c
                                nc.tensor.transpose(
                                    tph[:, c * P:(c + 1) * P],
                                    normed2[:, cc * P:(cc + 1) * P], ident)
                            if half == 0:
                                nc.scalar.copy(inp2T[:, :D], tph)
                            else:
                                nc.vector.tensor_copy(inp2T[:, D:], tph)
                    else:
                        tpt2 = tp.tile([P, D2], MMDT, tag="tp")
                        for c in range(KC):
                            cs = slice(c * P, (c + 1) * P)
                            nc.tensor.transpose(tpt2[:, cs], normed2[:, cs], ident)
                        nc.scalar.copy(inp2T[:, :D], tpt2[:, :D])
                        nc.vector.tensor_copy(inp2T[:, D:], tpt2[:, D:])

                    ups = mm.tile([P, D], F32, tag="mmtile")
                    if no_mm:
                        nc.vector.memset(ups[:, :1], 1.0)
                    if not no_mm:
                        for c in range(KC):
                            nc.tensor.matmul(
                                ups,
                                lhsT=inp2T[:, c * P:(c + 1) * P],
                                rhs=wut[:, c * D:(c + 1) * D],
                                start=(c == 0),
                                stop=(c == KC - 1 and (no_aug or dve_bias)),
                            )
                        if not no_aug and not dve_bias:
                            nc.tensor.matmul(ups, lhsT=ones_row, rhs=ub,
                                             start=False, stop=True)
                    u = work.tile([P, D], F32, tag="u")
                    if dve_bias:
                        ub_in = work.tile([P, D], F32, tag="ub_in")
                        nc.vector.tensor_tensor(ub_in, ups, ubb, op=OP.add)
                        nc.scalar.activation(u, ub_in, AF.Tanh)
                    else:
                        nc.scalar.activation(u, ups, AF.Tanh)

                    # ---- softmax denominator (gpsimd) + combine ----
                    s = work.tile([P, D], F32, tag="s")
                    nc.gpsimd.tensor_tensor(s, e[:, :D], e[:, D:2 * D], op=OP.add)
                    nc.gpsimd.tensor_tensor(s, s, e[:, 2 * D:], op=OP.add)
                    rs = work.tile([P, D], F32, tag="rs")
                    if no_recip:
                        nc.vector.tensor_copy(rs, s)
                    else:
                        nc.vector.reciprocal(rs, s)

                    m1 = work.tile([P, D], F32, tag="m1")
                    nc.vector.tensor_tensor(m1, xv, e[:, :D], op=OP.mult)
                    m2 = work.tile([P, D], F32, tag="m2")
                    nc.vector.tensor_tensor(m2, hv, e[:, D:2 * D], op=OP.mult)
                    m3 = work.tile([P, D], F32, tag="m3")
                    nc.gpsimd.tensor_tensor(m3, u, e[:, 2 * D:], op=OP.mult)
                    nc.vector.tensor_tensor(m1, m1, m2, op=OP.add)
                    nc.vector.tensor_tensor(m1, m1, m3, op=OP.add)
                    nc.vector.tensor_tensor(m1, m1, rs, op=OP.mult)
                    nc.sync.dma_start(out=out_d[rows, :], in_=m1)

    nc.compile()
    return nc


_CACHE = {}


def _prep_inputs(x, h, Wg, bg, Wu, bu, ln_w, ln_b, ln2_w, ln2_b, dropout_mask):
    f = lambda a: np.ascontiguousarray(np.asarray(a, dtype=np.float32))
    x, h, Wg, bg, Wu, bu = f(x), f(h), f(Wg), f(bg), f(Wu), f(bu)
    wm = f(ln_w) * f(dropout_mask)
    bm = f(ln_b) * f(dropout_mask)
    w2m = f(ln2_w) * f(dropout_mask)
    b2m = f(ln2_b) * f(dropout_mask)

    wgt = np.ascontiguousarray((Wg * wm[None, :]).T)           # [2d, 5d]
    bg_eff = bg + Wg @ bm                                      # [5d]
    wut = np.ascontiguousarray((Wu * w2m[None, :]).T)          # [2d, d]
    ub = (bu + Wu @ b2m)[None, :]                              # [1, d]

    # biases for the first 4 gate n-tiles; the z chunks get the third z bias
    # subtracted (softmax shift-invariance) so chunk 4 needs no bias at all.
    z2b = bg_eff[4 * D:5 * D] - GATE_BIAS
    gb = np.concatenate([
        bg_eff[0 * D:1 * D],
        bg_eff[1 * D:2 * D],
        bg_eff[2 * D:3 * D] - z2b,
        bg_eff[3 * D:4 * D] - z2b,
    ])[None, :]                                                # [1, 4d]
    return x, h, wgt, np.ascontiguousarray(gb), wut, np.ascontiguousarray(ub)


def kernel(x, h, Wg, bg, Wu, bu, ln_w, ln_b, ln2_w, ln2_b, dropout_mask):
    x, h, wgt, gb, wut, ub = _prep_inputs(
        x, h, Wg, bg, Wu, bu, ln_w, ln_b, ln2_w, ln2_b, dropout_mask)

    if "nc" not in _CACHE:
        _CACHE["nc"] = _build()
    nc = _CACHE["nc"]

    in_maps = [
        {"x": x[c * B_LOC:(c + 1) * B_LOC], "h": h[c * B_LOC:(c + 1) * B_LOC],
         "wgt": wgt, "gb": gb, "wut": wut, "ub": ub}
        for c in range(N_CORES)
    ]
    res = run_bass_kernel_spmd(nc, in_maps, list(range(N_CORES)))
    return np.concatenate([res.results[c]["out"] for c in range(N_CORES)], axis=0)

